# revision 1
# baseline (speedup 1.0000x reference)
"""MoE FFN kernel for Trainium2 (Bass/Tile), data-parallel over tokens on 8 cores.

Problem (hardcoded): B=4, S=2048, H=1024, R=256, F=4096, E=8, top-K=2.
  logits = x @ router_w + router_b            [T, E]
  rw     = softmax(logits);  mask = top2(rw)  (== top2 of logits)
  t_e    = (x @ w1_e) * (rw*mask)[:, e]       [T, R]  per expert
  mixed  = sum_e t_e @ w2_e                   [T, F]
  out    = gelu_erf(mixed) @ lin2_w + lin2_b  [T, H]

Device strategy (per core, T=1024 tokens, everything kept transposed
[feature, token] so no on-device transposes are needed):
  - router computed in ~fp32 via bf16 hi/lo split (xb@wb + xr@wb + xb@wr),
    logits live as [E, T] (experts on partitions); softmax/top-2 via
    gpsimd partition_all_reduce; routing weights mwT [E, T] f32.
  - tokens processed in two halves of 512 (SBUF pressure):
    per expert: tT_e [R, TH] = w1_e.T @ x (bf16 matmuls, f32 accum), scaled
    on PSUM-evacuation by mwb_e = partition_broadcast(mwT[e]) -> bf16;
    mixedT [F, TH] accumulated over (e, r) in PSUM; erf-Gelu on ScalarE
    evacuates to hT bf16; outT [H, TH] = lin2_w.T @ h + bias -> DMA out.
Host packs all tensors so every DMA is per-partition contiguous.
"""

import numpy as np
import ml_dtypes

P = 128
B, S, H, R, F, E = 4, 2048, 1024, 256, 4096, 8
NCORES = 8
TT = B * S              # 8192 tokens total
T = TT // NCORES        # 1024 tokens per core
TH = 512                # token half processed at once in phases 2-4
NH = T // TH            # 2
KO = H // P             # 8  (H chunks)
RO = R // P             # 2  (R chunks)
FT = F // P             # 32 (F tiles)
HT = H // P             # 8  (output H tiles)

_CACHE: dict = {}


def _build_nc(act_name: str = "Gelu"):
    import concourse.mybir as mybir
    import concourse.tile as tile
    from concourse import bacc, bass_isa

    dt = mybir.dt
    f32, bf16 = dt.float32, dt.bfloat16
    Alu = mybir.AluOpType
    Act = mybir.ActivationFunctionType

    nc = bacc.Bacc(
        "TRN2", target_bir_lowering=False, debug=False, enable_asserts=False
    )

    def din(name, shape, dtype):
        return nc.dram_tensor(name, shape, dtype, kind="ExternalInput").ap()

    xb = din("xb", [P, KO, T], bf16)        # x^T hi
    xr = din("xr", [P, KO, T], bf16)        # x^T residual
    wb = din("wb", [P, KO, E], bf16)        # router_w hi
    wr = din("wr", [P, KO, E], bf16)        # router_w residual
    rb = din("rb", [E, 1], f32)             # router_b
    w1 = din("w1", [P, E, KO, R], bf16)
    w2 = din("w2", [FT, P, E, RO, P], bf16)
    l2 = din("l2", [HT, P, FT, P], bf16)
    l2b = din("l2b", [P, HT], f32)
    out = nc.dram_tensor("o", [P, HT, T], f32, kind="ExternalOutput").ap()

    with tile.TileContext(nc) as tc:
        from contextlib import ExitStack

        with ExitStack() as ctx:
            res = ctx.enter_context(tc.tile_pool(name="res", bufs=1))
            smp = ctx.enter_context(tc.tile_pool(name="smp", bufs=1))
            mwbp = ctx.enter_context(tc.tile_pool(name="mwbp", bufs=2))
            mwsp = ctx.enter_context(tc.tile_pool(name="mwsp", bufs=2))
            w2p = ctx.enter_context(tc.tile_pool(name="w2p", bufs=3))
            l2p = ctx.enter_context(tc.tile_pool(name="l2p", bufs=2))
            outp = ctx.enter_context(tc.tile_pool(name="outp", bufs=2))
            psp = ctx.enter_context(tc.tile_pool(name="psp", bufs=4, space="PSUM"))
            pslp = ctx.enter_context(tc.tile_pool(name="pslp", bufs=1, space="PSUM"))

            # ---- resident loads (per-partition contiguous layouts) ----
            xb_sb = res.tile([P, KO, T], bf16)
            xr_sb = res.tile([P, KO, T], bf16)
            for k in range(KO):
                nc.sync.dma_start(xb_sb[:, k], xb[:, k])
                nc.sync.dma_start(xr_sb[:, k], xr[:, k])
            wb_sb = res.tile([P, KO, E], bf16)
            nc.sync.dma_start(wb_sb, wb)
            wr_sb = res.tile([P, KO, E], bf16)
            nc.sync.dma_start(wr_sb, wr)
            rb_sb = res.tile([E, 1], f32)
            nc.sync.dma_start(rb_sb, rb)
            w1_sb = res.tile([P, E, KO, R], bf16)
            for e in range(E):
                nc.sync.dma_start(w1_sb[:, e], w1[:, e])
            l2b_sb = res.tile([P, HT], f32)
            nc.sync.dma_start(l2b_sb, l2b)

            tT_sb = res.tile([P, E * RO, TH], bf16)
            hT_sb = res.tile([P, FT, TH], bf16)
            mwT_sb = res.tile([E, T], f32)

            # ---- router: logits [E, T] in ~fp32 via hi/lo bf16 ----
            psl = pslp.tile([E, T], f32)
            groups = [(xb_sb, wb_sb), (xr_sb, wb_sb), (xb_sb, wr_sb)]
            for gi, (xx, ww) in enumerate(groups):
                for k in range(KO):
                    for t in range(T // 512):
                        nc.tensor.matmul(
                            psl[:, t * 512:(t + 1) * 512],
                            ww[:, k, :],
                            xx[:, k, t * 512:(t + 1) * 512],
                            start=(gi == 0 and k == 0),
                            stop=(gi == 2 and k == KO - 1),
                        )
            # softmax + top-2 on [E, T]; scratch tiles reused carefully.
            lg = smp.tile([E, T], f32)
            nc.vector.tensor_scalar(lg, psl, rb_sb[:, 0:1], None, op0=Alu.add)
            m1 = smp.tile([E, T], f32)
            nc.gpsimd.partition_all_reduce(
                m1, lg, channels=E, reduce_op=bass_isa.ReduceOp.max
            )
            scr = smp.tile([E, T], f32)
            nc.vector.tensor_tensor(scr, lg, m1, Alu.is_equal)
            nc.vector.tensor_scalar(scr, scr, -1e30, None, op0=Alu.mult)
            nc.vector.tensor_tensor(scr, lg, scr, Alu.add)  # lg; argmax -> -inf
            m2 = smp.tile([E, T], f32)
            nc.gpsimd.partition_all_reduce(
                m2, scr, channels=E, reduce_op=bass_isa.ReduceOp.max
            )
            nc.vector.tensor_tensor(scr, lg, m2, Alu.is_ge)   # scr = top-2 mask
            nc.vector.tensor_tensor(lg, lg, m1, Alu.subtract)  # lg = lg - max
            nc.scalar.activation(m1, lg, Act.Exp)              # m1 = exp
            nc.gpsimd.partition_all_reduce(
                lg, m1, channels=E, reduce_op=bass_isa.ReduceOp.add
            )                                                  # lg = sum(exp)
            nc.vector.reciprocal(m2, lg)                       # m2 = 1/sum
            nc.vector.tensor_tensor(m1, m1, scr, Alu.mult)     # exp * mask
            nc.vector.tensor_tensor(mwT_sb, m1, m2, Alu.mult)  # mwT

            for th in range(NH):
                tsl = slice(th * TH, (th + 1) * TH)

                # ---- experts: tT[e*RO+r] = (w1_e.T @ x) * mw_e  (bf16) ----
                for e in range(E):
                    # move mw row e to partition 0, then broadcast to 128
                    mws = mwsp.tile([1, TH], f32)
                    nc.sync.dma_start(mws, mwT_sb[e:e + 1, tsl])
                    mwb = mwbp.tile([P, TH], f32)
                    nc.gpsimd.partition_broadcast(mwb, mws[0:1, :])
                    for r in range(RO):
                        pst = psp.tile([P, TH], f32, tag="ps")
                        for k in range(KO):
                            nc.tensor.matmul(
                                pst,
                                w1_sb[:, e, k, r * P:(r + 1) * P],
                                xb_sb[:, k, tsl],
                                start=(k == 0),
                                stop=(k == KO - 1),
                            )
                        nc.vector.tensor_tensor(
                            tT_sb[:, e * RO + r, :], pst, mwb, Alu.mult
                        )

                # ---- mixedT [F, TH] + erf-gelu -> hT bf16 ----
                for ft in range(FT):
                    w2t = w2p.tile([P, E, RO, P], bf16)
                    nc.sync.dma_start(w2t, w2[ft])
                    psf = psp.tile([P, TH], f32, tag="ps")
                    for e in range(E):
                        for r in range(RO):
                            nc.tensor.matmul(
                                psf,
                                w2t[:, e, r, :],
                                tT_sb[:, e * RO + r, :],
                                start=(e == 0 and r == 0),
                                stop=(e == E - 1 and r == RO - 1),
                            )
                    nc.scalar.activation(
                        hT_sb[:, ft, :], psf, getattr(Act, act_name)
                    )

                # ---- outT [H, TH] = lin2.T @ h + b ----
                for ht in range(HT):
                    l2t = l2p.tile([P, FT, P], bf16)
                    nc.sync.dma_start(l2t, l2[ht])
                    pso = psp.tile([P, TH], f32, tag="ps")
                    for ko in range(FT):
                        nc.tensor.matmul(
                            pso,
                            l2t[:, ko, :],
                            hT_sb[:, ko, :],
                            start=(ko == 0),
                            stop=(ko == FT - 1),
                        )
                    ot = outp.tile([P, TH], f32)
                    nc.vector.tensor_scalar(
                        ot, pso, l2b_sb[:, ht:ht + 1], None, op0=Alu.add
                    )
                    nc.sync.dma_start(out[:, ht, tsl], ot)

    nc.compile()
    return nc


def get_nc(act_name: str = "Gelu"):
    key = f"nc_{act_name}"
    if key not in _CACHE:
        _CACHE[key] = _build_nc(act_name)
    return _CACHE[key]


def pack_inputs(inputs):
    """Full-problem numpy inputs -> list of 8 per-core in_maps (packed)."""
    bf = ml_dtypes.bfloat16
    x = np.asarray(inputs["x"], np.float32).reshape(TT, H)
    router_w = np.asarray(inputs["router_w"], np.float32)
    router_b = np.asarray(inputs["router_b"], np.float32)
    w1 = np.asarray(inputs["w1"], np.float32)
    w2 = np.asarray(inputs["w2"], np.float32)
    lin2_w = np.asarray(inputs["lin2_w"], np.float32)
    lin2_b = np.asarray(inputs["lin2_b"], np.float32)

    wbh = router_w.astype(bf)
    wrr = (router_w - wbh.astype(np.float32)).astype(bf)
    # [H, E] -> [P, KO, E]
    wb_p = np.ascontiguousarray(wbh.reshape(KO, P, E).transpose(1, 0, 2))
    wr_p = np.ascontiguousarray(wrr.reshape(KO, P, E).transpose(1, 0, 2))
    rb_p = np.ascontiguousarray(router_b.reshape(E, 1).astype(np.float32))
    # [E, H, R] -> [P, E, KO, R]
    w1_p = np.ascontiguousarray(
        w1.astype(bf).reshape(E, KO, P, R).transpose(2, 0, 1, 3)
    )
    # [E, R, F] -> [FT, P, E, RO, C]
    w2_p = np.ascontiguousarray(
        w2.astype(bf).reshape(E, RO, P, FT, P).transpose(3, 2, 0, 1, 4)
    )
    # [F, H] -> [HT, P, FT, C]
    l2_p = np.ascontiguousarray(
        lin2_w.astype(bf).reshape(FT, P, HT, P).transpose(2, 1, 0, 3)
    )
    # [H] -> [P, HT]
    l2b_p = np.ascontiguousarray(
        lin2_b.astype(np.float32).reshape(HT, P).T
    )

    shared = {
        "wb": wb_p, "wr": wr_p, "rb": rb_p,
        "w1": w1_p, "w2": w2_p, "l2": l2_p, "l2b": l2b_p,
    }
    in_maps = []
    for c in range(NCORES):
        xt = x[c * T:(c + 1) * T].T  # [H, T]
        xtb = xt.astype(bf)
        xtr = (xt - xtb.astype(np.float32)).astype(bf)
        xb_p = np.ascontiguousarray(xtb.reshape(KO, P, T).transpose(1, 0, 2))
        xr_p = np.ascontiguousarray(xtr.reshape(KO, P, T).transpose(1, 0, 2))
        in_maps.append({"xb": xb_p, "xr": xr_p, **shared})
    return in_maps


def unpack_outputs(outs):
    """list of 8 per-core [P, HT, T] f32 -> [B, S, H] f32."""
    parts = []
    for o in outs:
        # o[p, ht, t] = out_core[t, ht*P + p]
        oc = np.asarray(o).transpose(2, 1, 0).reshape(T, H)
        parts.append(oc)
    return np.concatenate(parts, axis=0).reshape(B, S, H)


def kernel(**inputs) -> np.ndarray:
    from concourse import bass_utils

    nc = get_nc()
    in_maps = pack_inputs(inputs)
    res = bass_utils.run_bass_kernel_spmd(
        nc, in_maps, core_ids=list(range(NCORES))
    )
    return unpack_outputs([r["o"] for r in res.results])

